# revision 5
# baseline (speedup 1.0000x reference)
"""CRF head kernel for Trainium2 (Bass/Tile), 8-core data-parallel.

Computes: out[b, t, :] = x[b, t, :] + transitions[argmax(x[b, t, :]), :]
for x of shape [128, 1024, 256] f32 and transitions [256, 256] f32.

Sharding: batch dim split across 8 NeuronCores (16 batches / core).
Per core: 16*1024 = 16384 rows, processed in megatiles of P*G = 2048 rows
laid out as [128 partitions, 16 groups, 256 tags] (each partition holds 16
consecutive rows -> contiguous 16KB DMA descriptors per partition).

No dma_gather: the transitions row selection runs on the Tensor engine as
a one-hot matmul, so the DMA fabric carries only the compulsory traffic
(load x + store y = 33.6 MB/core).

Per megatile:
  1. HWDGE load 2MB from HBM (sync queue).
  2. DVE: reduce_max over tags -> mx [128, G]; per group c a max_index
     restricted to that group's 256-wide window (first-occurrence == argmax,
     no cross-group collisions); convert indices to bf16.
  3. GpSimd: one-hot oh[p, c, d] = (iota_d == idx[p, c]) in bf16.
  4. PE: transpose one-hot halves to [tag, row] (bf16, via identity),
     ACT copies PSUM->SBUF, then 2 accumulating matmuls against resident
     bf16 T_lo/T_hi -> PSUM holds transitions[argmax] [128 rows, 256].
  5. GpSimd: add x + PSUM in place; HWDGE store 2MB (scalar queue).
"""

import sys

for _p in ("/opt/trn_rl_repo",):
    if _p not in sys.path:
        sys.path.append(_p)

import numpy as np

import concourse.bass as bass
import concourse.bacc as bacc
import concourse.mybir as mybir
import concourse.tile as tile
import concourse.bass_utils as bass_utils
from concourse import masks

N_CORES = 8
B, T, TAGS = 128, 1024, 256
R = (B // N_CORES) * T          # rows per core = 16384
P = 128                         # SBUF partitions
G = 16                          # rows per partition per megatile

_CACHE = {}


def _build(rows=R, g=G):
    rows_per_mt = P * g
    n_mt = rows // rows_per_mt
    assert n_mt * rows_per_mt == rows

    nc = bacc.Bacc("TRN2", target_bir_lowering=False, debug=False)

    x = nc.dram_tensor("x", [rows, TAGS], mybir.dt.float32, kind="ExternalInput")
    t = nc.dram_tensor("t", [TAGS, TAGS], mybir.dt.float32, kind="ExternalInput")
    y = nc.dram_tensor("y", [rows, TAGS], mybir.dt.float32, kind="ExternalOutput")

    # megatile m, partition p holds rows m*rows_per_mt + p*g .. +g-1
    xv = x.ap().rearrange("(m p g) d -> m p (g d)", p=P, g=g)
    yv = y.ap().rearrange("(m p g) d -> m p (g d)", p=P, g=g)

    with tile.TileContext(nc) as tc:
        with (
            tc.tile_pool(name="cp", bufs=1) as cp,
            tc.tile_pool(name="xp", bufs=3) as xp,
            tc.tile_pool(name="ohp", bufs=2) as ohp,
            tc.tile_pool(name="wp", bufs=6) as wp,
            tc.tile_pool(name="sp", bufs=4) as sp,
            tc.tile_pool(name="tp", bufs=4, space="PSUM") as tp,
            tc.tile_pool(name="mp", bufs=3, space="PSUM") as mp,
        ):
            # ---- constants -------------------------------------------------
            ident = cp.tile([P, P], mybir.dt.bfloat16, tag="id", name="ident")
            masks.make_identity(nc, ident[:])

            iotu = cp.tile([P, TAGS], mybir.dt.uint16, tag="iu", name="iotu")
            nc.gpsimd.iota(iotu[:], pattern=[[1, TAGS]], base=0,
                           channel_multiplier=0)
            iotb = cp.tile([P, TAGS], mybir.dt.bfloat16, tag="ib", name="iotb")
            nc.vector.tensor_copy(iotb[:], iotu[:])

            # transitions resident in SBUF as bf16, split in two K-halves
            tf32 = cp.tile([P, 2 * TAGS], mybir.dt.float32, tag="tf", name="tf32")
            _tap = t.ap()
            tv = bass.AP(_tap.tensor, _tap.offset,
                         [[TAGS, P], [P * TAGS, 2], [1, TAGS]])
            nc.sync.dma_start(out=tf32[:], in_=tv)
            tbf = cp.tile([P, 2 * TAGS], mybir.dt.bfloat16, tag="tb", name="tbf")
            nc.vector.tensor_copy(tbf[:], tf32[:])
            t_lo = tbf[:, 0:TAGS]
            t_hi = tbf[:, TAGS:2 * TAGS]

            for m in range(n_mt):
                x_t = xp.tile([P, g * TAGS], mybir.dt.float32, tag="x",
                              name=f"x_{m}")
                nc.sync.dma_start(out=x_t[:], in_=xv[m])
                x3 = x_t[:].rearrange("p (c d) -> p c d", d=TAGS)

                mx = sp.tile([P, g], mybir.dt.float32, tag="mx", name=f"mx_{m}")
                nc.vector.tensor_reduce(out=mx[:], in_=x3,
                                        axis=mybir.AxisListType.X,
                                        op=mybir.AluOpType.max)

                idxw = sp.tile([P, g, 8], mybir.dt.uint16, tag="iw",
                               name=f"iw_{m}")
                for c in range(g):
                    nc.vector.max_index(
                        out=idxw[:, c, :],
                        in_max=mx[:, c:c + 1].to_broadcast([P, 8]),
                        in_values=x_t[:, c * TAGS:(c + 1) * TAGS],
                    )
                idxb = sp.tile([P, g], mybir.dt.bfloat16, tag="ix",
                               name=f"ix_{m}")
                nc.vector.tensor_copy(idxb[:], idxw[:, :, 0])

                oh = ohp.tile([P, g * TAGS], mybir.dt.bfloat16, tag="oh",
                              name=f"oh_{m}")
                oh3 = oh[:].rearrange("p (c d) -> p c d", d=TAGS)
                nc.vector.tensor_tensor(
                    out=oh3,
                    in0=iotb[:].rearrange("p d -> p () d").to_broadcast(
                        [P, g, TAGS]),
                    in1=idxb[:].to_broadcast([P, g, TAGS]),
                    op=mybir.AluOpType.is_equal,
                )

                # software-pipelined: transposes for group c run on PE while
                # ACT copies c-1's halves out of PSUM; matmuls trail by one c.
                pend = None

                def flush(pend):
                    w_lo, w_hi, c = pend
                    ps = mp.tile([P, TAGS], mybir.dt.float32, tag="ps",
                                 name=f"ps_{m}_{c}")
                    nc.tensor.matmul(ps[:], lhsT=w_lo[:], rhs=t_lo,
                                     start=True, stop=False)
                    nc.tensor.matmul(ps[:], lhsT=w_hi[:], rhs=t_hi,
                                     start=False, stop=True)
                    nc.vector.tensor_add(
                        out=x_t[:, c * TAGS:(c + 1) * TAGS],
                        in0=x_t[:, c * TAGS:(c + 1) * TAGS],
                        in1=ps[:])

                for c in range(g):
                    pt_lo = tp.tile([P, P], mybir.dt.bfloat16, tag="pt",
                                    name=f"ptl_{m}_{c}")
                    pt_hi = tp.tile([P, P], mybir.dt.bfloat16, tag="pt",
                                    name=f"pth_{m}_{c}")
                    nc.tensor.transpose(pt_lo[:], oh3[:, c, 0:P], ident[:])
                    nc.tensor.transpose(pt_hi[:], oh3[:, c, P:TAGS], ident[:])
                    w_lo = wp.tile([P, P], mybir.dt.bfloat16, tag="w",
                                   name=f"wl_{m}_{c}")
                    w_hi = wp.tile([P, P], mybir.dt.bfloat16, tag="w",
                                   name=f"wh_{m}_{c}")
                    nc.scalar.copy(w_lo[:], pt_lo[:])
                    nc.scalar.copy(w_hi[:], pt_hi[:])
                    if pend is not None:
                        flush(pend)
                    pend = (w_lo, w_hi, c)
                flush(pend)

                nc.scalar.dma_start(out=yv[m], in_=x_t[:])

    nc.compile()
    return nc


def get_nc():
    if "nc" not in _CACHE:
        _CACHE["nc"] = _build()
    return _CACHE["nc"]


def kernel(launch_matrix, transitions):
    launch = np.ascontiguousarray(np.asarray(launch_matrix, dtype=np.float32))
    trans = np.ascontiguousarray(np.asarray(transitions, dtype=np.float32))
    assert launch.shape == (B, T, TAGS), launch.shape
    assert trans.shape == (TAGS, TAGS), trans.shape

    nc = get_nc()
    shards = launch.reshape(N_CORES, R, TAGS)
    in_maps = [{"x": shards[c], "t": trans} for c in range(N_CORES)]
    res = bass_utils.run_bass_kernel_spmd(nc, in_maps,
                                          core_ids=list(range(N_CORES)))
    _CACHE["last_results"] = res
    out = np.concatenate([res.results[c]["y"] for c in range(N_CORES)], axis=0)
    return out.reshape(B, T, TAGS)
